# revision 1
# baseline (speedup 1.0000x reference)
"""Trainium2 Bass kernel for nn_LIFLayer (T=512, B=64, C_IN=C_OUT=512).

Strategy (data-parallel over batch, 8 batches/core, no collectives):

The reference per-step recurrence is restructured as:
  G_t   = x_t @ Wx^T + b                      (batched matmul, precomputed)
  S_t   = slow_{t-1} @ Ws^T + G_t             (sequential; 4 fp32r matmuls + I8 matmul)
  sig   = Sigmoid(S_t)                        (ACT)
  d_t   = 0.995^(0.9*sig+0.05) == Square(sc*sig + bs) + Delta   (exact to fp32, ACT)
  slow_t = d_t * slow_{t-1} + x_t             (DVE stt + add, channel-major)
  -- deferred, batched --
  fast_t = 0.9 fast_{t-1} + x_t               (native tensor_tensor_scan)
  z_t   = 2 x_t + fast_t + slow_t             (DVE; 0.5 and 0.1 folded into W01)
  cur_t = z_t @ (0.05 W)                      (batched fp32r matmul)
  v'    = 0.9 v + cur_t ; ns = (v' <= 1) ; acc += ns ; v = v' * ns
  out   = 1 - acc / T

Channel-major state layout [128c x (4k,8b)]; sigmoid output is transposed
back to channel-major with 4 tiny PE transposes per step.
"""

import math
import numpy as np

T, B, C = 512, 64, 512
CO = 512
NCORES = 8
BL = B // NCORES  # 8 batches per core
ALPHA = 0.9
A_FAST = 0.9
A_SLOW = 0.995

# quadratic expansion of d = A_SLOW**(0.9*sig + 0.05) = a0 + a1*sig + a2*sig^2
_L = math.log(A_SLOW)
_a0 = 1.0 + 0.05 * _L + 0.00125 * _L * _L
_a1 = 0.9 * _L + 0.045 * _L * _L
_a2 = 0.405 * _L * _L
SC = math.sqrt(_a2)
BS = _a1 / (2.0 * SC)
DELTA = _a0 - BS * BS

_NC_CACHE = {}


def build_nc(t_steps=T):
    import concourse.bass as bass
    import concourse.bacc as bacc
    import concourse.mybir as mybir
    from concourse.tile import TileContext
    from contextlib import ExitStack

    f32 = mybir.dt.float32
    f32r = mybir.dt.float32r
    AF = mybir.ActivationFunctionType
    OP = mybir.AluOpType

    NCH = t_steps // 16          # 16-tick chunks
    NDEF = t_steps // 64         # 64-tick deferred chunks

    nc = bacc.Bacc()

    seq_l = nc.dram_tensor("seq_l", [t_steps, BL, C], f32, kind="ExternalInput")
    wsT_d = nc.dram_tensor("wsT", [C, C], f32r, kind="ExternalInput")
    wxT_d = nc.dram_tensor("wxT", [C, C], f32r, kind="ExternalInput")
    w01_d = nc.dram_tensor("w01", [C, CO], f32r, kind="ExternalInput")
    bias_d = nc.dram_tensor("biasv", [1, C], f32r, kind="ExternalInput")
    eye8_d = nc.dram_tensor("eye8", [8, 8], f32r, kind="ExternalInput")
    eye128_d = nc.dram_tensor("eye128", [128, 128], f32, kind="ExternalInput")
    ones_d = nc.dram_tensor("ones1", [1, 128], f32r, kind="ExternalInput")
    out_d = nc.dram_tensor("out_l", [BL, CO], f32, kind="ExternalOutput")

    def r(ap):  # matmul operands are already float32r-typed
        return ap

    with TileContext(nc) as tc, ExitStack() as ctx:
        dram = ctx.enter_context(tc.tile_pool(name="dram", bufs=1, space="DRAM"))
        xt_dram = dram.tile([128, 4, BL, t_steps], f32r)   # [c-part][k][b][t]
        slow_dram = dram.tile([128, 4, BL, t_steps], f32r)
        g_dram = dram.tile([NCH, 128, CO], f32r)     # [chunk][(16t,8b)][c]

        consts = ctx.enter_context(tc.tile_pool(name="consts", bufs=1))
        wsT_sb = consts.tile([128, 4, C], f32r)
        wxT_sb = consts.tile([128, 4, C], f32r)
        w01_sb = consts.tile([128, 4, CO], f32r)
        bias_sb = consts.tile([1, C], f32r)
        eye8_sb = consts.tile([8, 8], f32r)
        eye128_sb = consts.tile([128, 128], f32)
        ones_sb = consts.tile([1, 128], f32r)
        zeros32 = consts.tile([128, 4, 8], f32)
        c09 = consts.tile([128, 64], f32)
        bs_ap = consts.tile([128, 1], f32)
        nc.vector.memset(bs_ap, BS)

        nc.sync.dma_start(wsT_sb, wsT_d.rearrange("(k p) j -> p k j", p=128))
        nc.sync.dma_start(wxT_sb, wxT_d.rearrange("(k p) j -> p k j", p=128))
        nc.sync.dma_start(w01_sb, w01_d.rearrange("(k p) j -> p k j", p=128))
        nc.sync.dma_start(bias_sb, bias_d[:, :])
        nc.sync.dma_start(eye8_sb, eye8_d[:, :])
        nc.sync.dma_start(eye128_sb, eye128_d[:, :])
        nc.sync.dma_start(ones_sb, ones_d[:, :])
        nc.vector.memset(zeros32, 0.0)
        nc.vector.memset(c09, A_FAST)

        # ---------------- phase 1: gate recurrence (fused G precompute) -----
        small = ctx.enter_context(tc.tile_pool(name="small", bufs=2))
        phase1 = ExitStack()
        gate = phase1.enter_context(tc.tile_pool(name="gate", bufs=3))
        gpsum = phase1.enter_context(tc.tile_pool(name="gpsum", bufs=2, space="PSUM"))
        gstage = phase1.enter_context(tc.tile_pool(name="gstage", bufs=8))

        prev_slow = None  # AP of slowT(t-1)  [128,(4,8)] view provider
        for u in range(NCH):
            # --- chunk prep: load seq, transpose to channel-major, G matmul
            seqc = gate.tile([128, C], f32, tag="seqc")
            nc.sync.dma_start(
                seqc, seq_l[u * 16:(u + 1) * 16].rearrange("t b c -> (t b) c")
            )
            xt_ps = gpsum.tile([128, 4, BL, 16], f32, tag="xt_ps")
            for k in range(4):
                nc.tensor.transpose(
                    xt_ps[:, k].rearrange("p b t -> p t b"),
                    seqc[:, k * 128:(k + 1) * 128],
                    eye128_sb,
                )
            xt_c = gate.tile([128, 4, BL, 16], f32r, tag="xt_c")
            nc.scalar.activation(
                xt_c.rearrange("p k b t -> p (k b t)"),
                xt_ps.rearrange("p k b t -> p (k b t)"),
                AF.Copy,
            )
            nc.sync.dma_start(xt_dram[:, :, :, 16 * u:16 * (u + 1)], xt_c)

            g_ps = gpsum.tile([128, CO], f32, tag="g_ps")
            for k in range(4):
                nc.tensor.matmul(
                    g_ps,
                    r(xt_c[:, k].rearrange("p b t -> p (b t)")),
                    r(wxT_sb[:, k, :]),
                    start=(k == 0),
                    stop=False,
                )
            nc.tensor.matmul(g_ps, r(ones_sb), r(bias_sb), start=False, stop=True)
            g_c = gate.tile([128, CO], f32r, tag="g_c")
            nc.scalar.activation(g_c, g_ps, AF.Copy)
            nc.sync.dma_start(g_dram[u], g_c)

            slow_c = gate.tile([128, 4, BL, 16], f32r, tag="slow_c")

            # --- 16 sequential ticks
            for j in range(16):
                tau = u * 16 + j
                S = gpsum.tile([8, 512], f32, tag="S")
                if tau > 0:
                    for k in range(4):
                        nc.tensor.matmul(
                            S,
                            r(prev_slow[:, k]),
                            r(wsT_sb[:, k, :]),
                            start=(k == 0),
                            stop=False,
                        )
                gt = gstage.tile([8, 512], f32r, tag="gt")
                nc.sync.dma_start(
                    gt, g_dram[u].rearrange("(b t) c -> t b c", t=16)[j]
                )
                nc.tensor.matmul(
                    S,
                    r(eye8_sb),
                    r(gt),
                    start=(tau == 0),
                    stop=True,
                )
                sig = small.tile([8, 512], f32, tag="sig")
                nc.scalar.activation(sig, S, AF.Sigmoid)
                sigT = gpsum.tile([128, 4, 8], f32, tag="sigT")
                for k in range(4):
                    nc.tensor.transpose(
                        sigT[:, k, :], sig[:, k * 128:(k + 1) * 128],
                        eye128_sb[0:8, 0:8],
                    )
                q = small.tile([128, 4, 8], f32, tag="q")
                nc.scalar.activation(
                    q[:, :, :],
                    sigT[:, :, :],
                    AF.Square,
                    bias=bs_ap,
                    scale=SC,
                )
                p_t = small.tile([128, 4, 8], f32, tag="p_t")
                nc.vector.scalar_tensor_tensor(
                    p_t[:, :, :],
                    q[:, :, :],
                    DELTA,
                    (prev_slow if tau > 0 else zeros32[:, :, :]),
                    op0=OP.add,
                    op1=OP.mult,
                )
                nc.vector.tensor_tensor(
                    slow_c[:, :, :, j],
                    p_t[:, :, :],
                    xt_c[:, :, :, j],
                    op=OP.add,
                )
                prev_slow = slow_c[:, :, :, j]

            nc.sync.dma_start(slow_dram[:, :, :, 16 * u:16 * (u + 1)], slow_c)

        # ---------------- phase 2: deferred fast/z/cur/v ---------------------
        phase1.close()
        dpool = ctx.enter_context(tc.tile_pool(name="dpool", bufs=2))
        dpsum = ctx.enter_context(tc.tile_pool(name="dpsum", bufs=1, space="PSUM"))
        state = ctx.enter_context(tc.tile_pool(name="state", bufs=1))

        v_st = state.tile([128, 4, 8], f32)
        acc = state.tile([128, 4, 8], f32)
        nc.vector.memset(v_st, 0.0)
        nc.vector.memset(acc, 0.0)

        prev_fast = None
        for w in range(NDEF):
            xt64 = dpool.tile([128, 4, BL, 64], f32r, tag="xt64")
            slow64 = dpool.tile([128, 4, BL, 64], f32r, tag="slow64")
            nc.sync.dma_start(xt64, xt_dram[:, :, :, 64 * w:64 * (w + 1)])
            nc.sync.dma_start(slow64, slow_dram[:, :, :, 64 * w:64 * (w + 1)])
            fast64 = dpool.tile([128, 4, BL, 64], f32, tag="fast64")
            for k in range(4):
                for b in range(BL):
                    nc.vector.tensor_tensor_scan(
                        fast64[:, k, b, :],
                        c09,
                        xt64[:, k, b, :],
                        initial=(
                            0.0 if w == 0 else prev_fast[:, k, b, 63:64]
                        ),
                        op0=OP.mult,
                        op1=OP.add,
                    )
            # z = 2x + fast + slow   (in place over xt64)
            zf = xt64.rearrange("p k b t -> p (k b t)")
            nc.vector.scalar_tensor_tensor(
                zf, zf, 2.0, fast64.rearrange("p k b t -> p (k b t)"),
                op0=OP.mult, op1=OP.add,
            )
            nc.gpsimd.tensor_tensor(
                zf, zf, slow64.rearrange("p k b t -> p (k b t)"), op=OP.add
            )
            cur_ps = dpsum.tile([128, 4, BL, 64], f32, tag="cur")
            for m in range(4):
                for k in range(4):
                    nc.tensor.matmul(
                        cur_ps[:, m].rearrange("p b t -> p (b t)"),
                        r(w01_sb[:, k, m * 128:(m + 1) * 128]),
                        r(xt64[:, k].rearrange("p b t -> p (b t)")),
                        start=(k == 0),
                        stop=(k == 3),
                    )
            vp64 = dpool.tile([128, 4, 8, 64], f32, tag="vp64")
            for t in range(64):
                nc.vector.scalar_tensor_tensor(
                    vp64[:, :, :, t],
                    v_st[:, :, :],
                    ALPHA,
                    cur_ps[:, :, :, t],
                    op0=OP.mult,
                    op1=OP.add,
                )
                nc.vector.scalar_tensor_tensor(
                    v_st[:, :, :],
                    vp64[:, :, :, t],
                    1.0,
                    vp64[:, :, :, t],
                    op0=OP.is_le,
                    op1=OP.mult,
                )
            # batched spike counting for the whole chunk
            ns64 = dpool.tile([128, 4, 8, 64], f32, tag="ns64")
            nc.vector.tensor_scalar(
                ns64.rearrange("p m b t -> p (m b t)"),
                vp64.rearrange("p m b t -> p (m b t)"),
                1.0,
                None,
                op0=OP.is_le,
            )
            nsum = small.tile([128, 4, 8], f32, tag="nsum")
            nc.vector.tensor_reduce(
                nsum[:, :, :], ns64[:, :, :, :], axis=mybir.AxisListType.X,
                op=OP.add,
            )
            nc.vector.tensor_tensor(
                acc[:, :, :], acc[:, :, :], nsum[:, :, :], op=OP.add
            )
            prev_fast = fast64

        res = state.tile([128, 4, 8], f32)
        nc.vector.tensor_scalar(
            res.rearrange("p m b -> p (m b)"),
            acc.rearrange("p m b -> p (m b)"),
            -1.0 / t_steps,
            1.0,
            op0=OP.mult,
            op1=OP.add,
        )
        resT_ps = dpsum.tile([8, 4, 128], f32)
        for m in range(4):
            nc.tensor.transpose(resT_ps[:, m, :], res[:, m, :], eye128_sb)
        resT = state.tile([8, 4, 128], f32)
        nc.scalar.activation(
            resT.rearrange("b m p -> b (m p)"),
            resT_ps.rearrange("b m p -> b (m p)"),
            AF.Copy,
        )
        nc.sync.dma_start(out_d[:, :], resT.rearrange("b m p -> b (m p)"))

    nc.finalize()
    return nc


def _prep_shared(seq, W, ctrl_w, ctrl_b):
    f = np.float32
    wsT = np.ascontiguousarray(ctrl_w[:, C:].T, dtype=f)
    wxT = np.ascontiguousarray(ctrl_w[:, :C].T, dtype=f)
    w01 = np.ascontiguousarray((1.0 - ALPHA) * 0.5 * W, dtype=f)
    bias = np.ascontiguousarray(ctrl_b[None, :], dtype=f)
    eye8 = np.eye(8, dtype=f)
    eye128 = np.eye(128, dtype=f)
    ones1 = np.ones((1, 128), dtype=f)
    return dict(wsT=wsT, wxT=wxT, w01=w01, biasv=bias, eye8=eye8,
                eye128=eye128, ones1=ones1)


LAST_EXEC_NS = None


def kernel(seq, W, ctrl_w, ctrl_b):
    global LAST_EXEC_NS
    import os
    from concourse.bass_utils import run_bass_kernel_spmd

    seq = np.asarray(seq, dtype=np.float32)
    t_steps = seq.shape[0]
    if t_steps not in _NC_CACHE:
        _NC_CACHE[t_steps] = build_nc(t_steps)
    nc = _NC_CACHE[t_steps]

    shared = _prep_shared(seq, np.asarray(W), np.asarray(ctrl_w),
                          np.asarray(ctrl_b))
    in_maps = []
    for c in range(NCORES):
        m = dict(shared)
        m["seq_l"] = np.ascontiguousarray(seq[:, c * BL:(c + 1) * BL, :])
        in_maps.append(m)

    trace = bool(os.environ.get("KERNEL_TRACE"))
    results = run_bass_kernel_spmd(
        nc, in_maps, core_ids=list(range(NCORES)), trace=trace
    )
    LAST_EXEC_NS = results.exec_time_ns
    return np.concatenate([res["out_l"] for res in results.results], axis=0)


if __name__ == "__main__":
    import reference

    inputs = {k: np.asarray(v) for k, v in reference.setup_inputs().items()}
    out = kernel(**inputs)
    print("kernel output", out.shape, out.dtype, out.mean())

